# revision 58
# baseline (speedup 1.0000x reference)
"""Trainium2 Bass kernel for nn_CausalSelfAttention_77695958385275.

Self-contained: hardcodes shapes/sharding from the problem spec.

Architecture (8 NeuronCores, tensor-parallel over heads, SPMD-homogeneous):
  core c owns: dense head c, sparse head 8+c, full KV head c//2 (for the
  dense head), strided-only KV head 4+c//2 (for the sparse head).
  Every core runs the identical program; only input data differs.

v2 vs v1: bf16 attention operands (kT/q/v/P — halves LDWEIGHTS, 1c/r
masks), softmax denominators accumulated on DVE instead of per-tile
ones-matmuls (cuts 1/3 of dense-attention PE streams), Shared-output
AllToAll, warmup collective removed (b0's A2A absorbs cold-start off
the critical path), merged startup scopes, per-ci ya loads on the idle
sync queue (kills the scalar-queue head-of-line stall before the b1
projection).
"""

import math
import ml_dtypes
import numpy as np

import bass_rust
import concourse.bass as bass
import concourse.tile as tile
from concourse import mybir
from concourse.bass_utils import run_bass_kernel_spmd
from concourse.tile import TileContext

# ---------------- problem constants ----------------
B, T, DIM = 2, 2048, 2048
H, KV, HD = 16, 8, 128
NUM_FULL = 8
STRIDE = 45
NS = (T + STRIDE - 1) // STRIDE  # 46 strided keys per batch
SCALE = 1.0 / np.sqrt(np.float32(HD)).astype(np.float32)
N_CORES = 8
BT = B * T  # 4096 tokens total
HALF = HD // 2

F32 = mybir.dt.float32
F32R = mybir.dt.float32r
BF16 = mybir.dt.bfloat16

QCH = 512            # attention q-chunk width
NTCH = T // QCH      # 4 q-chunks per batch
KTILE = 128          # key tile
XCH = 1024           # qkv token chunk (2KB DMA lines)
CT = DIM // 128      # 16 contraction tiles
TSL = T // N_CORES   # 256 tokens per rank per batch

ScopedClock = bass_rust.ScopedClock


class SplitDrainTileContext(TileContext):
    """This walrus build allows a single sync-wait slot per CTRL/drain;
    split the tail drain's waits across a chain of single-wait drains."""

    def _drain_and_barrier(self, tick_clock, wait_clock):
        nc = self.nc
        drain_inst = nc.sync.drain()
        wait_clock.add_sem_waits(
            drain_inst.ins, ScopedClock({None: tick_clock.global_clock})
        )
        si = drain_inst.ins.sync_info
        ow = list(si.on_wait or []) if si is not None else []
        if len(ow) > 1:
            si.on_wait = [ow[0]]
            drain_inst.ins.sync_info = si
            for w in ow[1:]:
                d2 = nc.sync.drain()
                s2 = d2.ins.sync_info
                if s2 is None:
                    s2 = bass_rust.SyncInfo(on_wait=[w], on_update=[])
                else:
                    s2.on_wait = [w]
                d2.ins.sync_info = s2
        nc.all_engine_barrier()
        assert self.sems is not None
        popped = nc._tile_sem_poison_stack.pop()
        assert popped is self._sem_poison
        nc.clear_and_free_semaphores(list(self.sems.allocated().values()))
        nc.all_engine_barrier()


def split_multi_waits(nc, max_waits=1):
    """Walrus here rejects >1 sync wait on several instruction formats; move
    extra waits onto preceding same-engine NoOps."""
    for f in nc.m.functions:
        for b in f.blocks:
            new = []
            changed = False
            for inst in b.instructions:
                si = inst.sync_info
                ow = list(si.on_wait) if (si is not None and si.on_wait) else []
                if len(ow) > max_waits:
                    changed = True
                    for w in ow[:-max_waits]:
                        nop = mybir.InstNoOp(
                            name=nc.get_next_instruction_name(), ins=[], outs=[]
                        )
                        nop.engine = inst.engine
                        nop.sync_info = bass_rust.SyncInfo(on_wait=[w], on_update=[])
                        new.append(nop)
                    si.on_wait = ow[-max_waits:]
                    inst.sync_info = si
                new.append(inst)
            if changed:
                b.instructions = new


# ---------------- host-side constant tables ----------------

def _rope_tables():
    pos = np.arange(T, dtype=np.float32)
    freqs = (np.arange(HALF, dtype=np.float32) / np.float32(HALF)).astype(np.float32)
    ang = pos[:, None] * freqs[None, :]          # [T, 64] f32
    cosv = np.cos(ang.astype(np.float64)).astype(np.float32).T   # [64, T]
    sinv = np.sin(ang.astype(np.float64)).astype(np.float32).T
    cc = np.concatenate([cosv, cosv], axis=0)    # [128, T]
    ss = np.concatenate([sinv, sinv], axis=0)
    ccT = np.concatenate([cc, cc], axis=1)       # [128, 4096] (b0|b1)
    ssT = np.concatenate([ss, ss], axis=1)
    sp = np.arange(0, T, STRIDE)
    ccS = np.concatenate([cc[:, sp], cc[:, sp]], axis=1)  # [128, 92]
    ssS = np.concatenate([ss[:, sp], ss[:, sp]], axis=1)
    return ccT, ssT, ccS, ssS


def _const_tables():
    BF = ml_dtypes.bfloat16
    ccT, ssT, ccS, ssS = _rope_tables()
    mrotT = np.zeros((HD, HD), np.float32)
    for i in range(HALF):
        mrotT[i + HALF, i] = -1.0   # (M^T)[i+64, i]: out[0:64] = -q[64:128]
        mrotT[i, i + HALF] = 1.0    # out[64:128] = +q[0:64]
    ident = np.eye(128, dtype=np.float32)
    ones = np.ones((128, 128), np.float32)
    # additive causal masks: 0 where valid, -1e9 where masked (added to
    # scores in PSUM via an identity-lhsT matmul; exp then yields 0)
    tri = np.where(np.arange(128)[None, :] >= np.arange(128)[:, None],
                   0.0, -1e9).astype(np.float32)          # [jk, x]
    q = np.arange(T)
    smask = np.where(q[None, :] >= (STRIDE * np.arange(NS))[:, None],
                     0.0, -1e9).astype(np.float32)        # [46, T]
    cast = lambda a: np.ascontiguousarray(a.astype(BF))
    return (cast(ccT), cast(ssT), cast(ccS), cast(ssS), cast(mrotT),
            cast(ident), np.ascontiguousarray(ones), cast(ones), cast(tri),
            cast(smask))


# ---------------- device program ----------------

def build_program():
    nc = bass.Bass(num_devices=N_CORES)

    # weights host-packed to [128, CT, M]: straight partition-major DMAs with
    # multi-KB contiguous lines (the [DIM, M] rearrange form had 512B lines)
    xT = nc.dram_tensor("xT", [DIM, BT], BF16, kind="ExternalInput")
    xsT = nc.dram_tensor("xsT", [128, CT, B * NS], BF16, kind="ExternalInput")
    wqT = nc.dram_tensor("wqT", [128, CT, 2 * HD], BF16, kind="ExternalInput")
    wkT = nc.dram_tensor("wkT", [128, CT, HD], BF16, kind="ExternalInput")
    wvT = nc.dram_tensor("wvT", [128, CT, HD], BF16, kind="ExternalInput")
    wksT = nc.dram_tensor("wksT", [128, CT, HD], BF16, kind="ExternalInput")
    wvsT = nc.dram_tensor("wvsT", [128, CT, HD], BF16, kind="ExternalInput")
    wpT = nc.dram_tensor("wpT", [128, CT, DIM], BF16, kind="ExternalInput")
    # token-sharded projection: each core ends up with a 256-token slice per
    # batch; host assembles by token slices
    outT = nc.dram_tensor("outT", [DIM, B * TSL], BF16, kind="ExternalOutput")
    wu_in = nc.dram_tensor("wu_in", [64, 64], BF16, kind="Internal")
    wu_out = nc.dram_tensor("wu_out", [64, 64], BF16, kind="Internal")
    wu2_in = nc.dram_tensor("wu2_in", [64, 64], BF16, kind="Internal")
    wu2_out = nc.dram_tensor("wu2_out", [64, 64], BF16, kind="Internal")

    # AllToAll per batch: in rows = 8 blocks of [dense128|sparse128] per
    # destination rank; out rows = same blocks from each source rank
    a2ain = [nc.dram_tensor(f"a2ain{b}", [N_CORES * 2 * HD, TSL], BF16,
                            kind="Internal") for b in range(B)]
    a2aout = [nc.dram_tensor(f"a2aout{b}", [N_CORES * 2 * HD, TSL], BF16,
                             kind="Internal") for b in range(B)]

    (ccT_h, ssT_h, ccS_h, ssS_h, mrotT_h, ident_h, onesf_h, onesb_h,
     tri_h, smask_h) = _const_tables()
    ccT_d = nc.inline_tensor(ccT_h, "ccT")
    ssT_d = nc.inline_tensor(ssT_h, "ssT")
    ccS_d = nc.inline_tensor(ccS_h, "ccS")
    ssS_d = nc.inline_tensor(ssS_h, "ssS")
    mrotT_d = nc.inline_tensor(mrotT_h, "mrotT")
    ident_d = nc.inline_tensor(ident_h, "ident")
    onesb_d = nc.inline_tensor(onesb_h, "onesb")
    tri_d = nc.inline_tensor(tri_h, "trim")
    smask_d = nc.inline_tensor(smask_h, "smask")

    AF = mybir.ActivationFunctionType
    OP = mybir.AluOpType

    with SplitDrainTileContext(nc) as tc:
        with tc.tile_pool(name="persist", bufs=1) as PP:
            # persistent SBUF state (bf16 except the f32r ones for sums)
            qdT = PP.tile([128, BT], BF16, tag="qdT")    # dense-head q^T (roped in place)
            qsT = PP.tile([128, BT], BF16, tag="qsT")    # sparse-head q^T
            kT = PP.tile([128, BT], BF16, tag="kT")      # full k^T of kv_a
            vtok = PP.tile([128, BT], BF16, tag="vtok")  # v token-major, 32 tiles of [128t,128d]
            ksT = PP.tile([128, B * NS], BF16, tag="ksT")     # strided k^T of kv_b
            vs = PP.tile([NS, B * HD], BF16, tag="vs")        # strided v token-major [46, (b,d)]
            mrot = PP.tile([128, 128], BF16, tag="mrot")
            identb = PP.tile([128, 128], BF16, tag="identb")
            onesb = PP.tile([128, 128], BF16, tag="onesb")
            tri = PP.tile([128, 128], BF16, tag="tri")
            smask = PP.tile([NS, T], BF16, tag="smask")
            ccS = PP.tile([128, B * NS], BF16, tag="ccS")
            ssS = PP.tile([128, B * NS], BF16, tag="ssS")

            # ------- Phase 1: strided k/v + QKV with per-chunk fused RoPE ----
            with tc.tile_pool(name="wstr", bufs=1) as WS, \
                 tc.tile_pool(name="wq", bufs=1) as WQ, \
                 tc.tile_pool(name="xs", bufs=2) as XS, \
                 tc.tile_pool(name="rtmp", bufs=3) as RT, \
                 tc.tile_pool(name="vtmp", bufs=2) as VT, \
                 tc.tile_pool(name="qkps", bufs=4, space="PSUM") as QPS, \
                 tc.tile_pool(name="trps", bufs=2, space="PSUM") as TPS, \
                 tc.tile_pool(name="rps", bufs=2, space="PSUM") as RPS:
                # startup-critical DMAs first: small weights for 1a + 1b
                xs_sb = WS.tile([128, CT, B * NS], BF16, tag="xs")
                wks_sb = WS.tile([128, CT, HD], BF16, tag="wks")
                wvs_sb = WS.tile([128, CT, HD], BF16, tag="wvs")
                wq_sb = WQ.tile([128, CT, 2 * HD], BF16, tag="wq")
                wk_sb = WQ.tile([128, CT, HD], BF16, tag="wk")
                wv_sb = WQ.tile([128, CT, HD], BF16, tag="wv")
                cc = WQ.tile([128, BT], BF16, tag="cc")
                ss = WQ.tile([128, BT], BF16, tag="ss")
                for ci in range(CT):
                    nc.sync.dma_start(wq_sb[:, ci, :], wqT[:, ci, :])
                nc.sync.dma_start(wk_sb[:], wkT[:])
                nc.sync.dma_start(wv_sb[:], wvT[:])
                nc.sync.dma_start(xs_sb[:], xsT[:])
                nc.sync.dma_start(wks_sb[:], wksT[:])
                nc.sync.dma_start(wvs_sb[:], wvsT[:])
                nc.sync.dma_start(mrot[:], mrotT_d[:])
                nc.sync.dma_start(ccS[:], ccS_d[:])
                nc.sync.dma_start(ssS[:], ssS_d[:])

                def phase_1a():
                    # strided k/v for the sparse kv head; emitted between QKV
                    # chunks 0 and 1 so it fills a PE bubble instead of
                    # gating the first QKV chunk on its input DMAs
                    ps_ks = QPS.tile([128, 512], F32, tag="mm", name="ps_ks")
                    for ci in range(CT):
                        nc.tensor.matmul(ps_ks[:, 0:B * NS], wks_sb[:, ci, :],
                                         xs_sb[:, ci, :],
                                         start=(ci == 0), stop=(ci == CT - 1))
                    nc.scalar.copy(ksT[:], ps_ks[:, 0:B * NS])
                    for b in range(B):
                        psv = QPS.tile([128, 512], F32, tag="mm", name="psv")
                        for ci in range(CT):
                            nc.tensor.matmul(
                                psv[0:NS, 0:HD],
                                xs_sb[:, ci, b * NS:(b + 1) * NS],
                                wvs_sb[:, ci, :],
                                start=(ci == 0), stop=(ci == CT - 1))
                        nc.vector.tensor_copy(vs[:, b * HD:(b + 1) * HD],
                                              psv[0:NS, 0:HD])
                    # strided k rope
                    rsw_sp = RPS.tile([128, 512], F32, tag="rsw", name="rsw_sp")
                    nc.tensor.matmul(rsw_sp[:, 0:B * NS], mrot[:], ksT[:],
                                     start=True, stop=True)
                    t1s = RT.tile([128, XCH], F32, tag="t1", name="t1s")
                    nc.gpsimd.tensor_mul(t1s[:, 0:B * NS], ksT[:], ccS[:])
                    t2s = RT.tile([128, XCH], F32, tag="t2", name="t2s")
                    nc.vector.scalar_tensor_tensor(
                        t2s[:, 0:B * NS], rsw_sp[:, 0:B * NS], 1.0, ssS[:],
                        op0=OP.mult, op1=OP.mult)
                    nc.vector.tensor_add(ksT[:], t1s[:, 0:B * NS],
                                         t2s[:, 0:B * NS])

                # ---- phase 1b: QKV chunk loop ----
                for tch in range(BT // XCH):
                    if tch == 1:
                        phase_1a()
                    c0 = tch * XCH
                    sl = slice(c0, c0 + XCH)
                    x_sb = XS.tile([128, CT, XCH], BF16, tag="x")
                    # per-ci pieces (2KB lines): accumulation groups chase the
                    # DMA instead of waiting for the whole chunk
                    for ci in range(CT):
                        nc.sync.dma_start(
                            x_sb[:, ci, :], xT[ci * 128:(ci + 1) * 128, sl])
                    nc.sync.dma_start(cc[:, sl], ccT_d[:, sl])
                    nc.sync.dma_start(ss[:, sl], ssT_d[:, sl])
                    if tch == 0:
                        # attention-phase constants: queue behind chunk 0 so
                        # they don't delay the startup-critical loads above
                        nc.sync.dma_start(identb[:], ident_d[:])
                        nc.sync.dma_start(onesb[:], onesb_d[:])
                        nc.sync.dma_start(tri[:], tri_d[:])
                        nc.sync.dma_start(smask[:], smask_d[:])

                    if tch == 1:
                        # tiny warmup AllToAll, triggered mid-QKV: acts as a
                        # cross-rank barrier so the first real A2A sees little
                        # rank skew; completes long before phase 1 drains
                        nc.gpsimd.collective_compute(
                            "AllToAll", OP.bypass,
                            ins=[wu_in[:]], outs=[wu_out[:]],
                            replica_groups=[list(range(N_CORES))],
                        )
                    if tch == BT // XCH - 1:
                        # second barrier, data-gated to fire near QKV end
                        # (~25us before the first real A2A): re-syncs the rank
                        # skew that re-accumulates over the 150us QKV phase
                        # and otherwise inflates the b0 A2A by 10-20us
                        nc.sync.dma_start(wu2_in[:], qdT[0:64, 0:64])
                        nc.gpsimd.collective_compute(
                            "AllToAll", OP.bypass,
                            ins=[wu2_in[:]], outs=[wu2_out[:]],
                            replica_groups=[list(range(N_CORES))],
                        )
                    # q0, q1, k -> evict -> rope in place (two 512 halves)
                    for mi, (wt, msl, dst) in enumerate((
                            (wq_sb, slice(0, 128), qdT),
                            (wq_sb, slice(128, 256), qsT),
                            (wk_sb, slice(0, 128), kT))):
                        for h in range(XCH // 512):
                            hs = slice(h * 512, (h + 1) * 512)
                            dsl = slice(c0 + h * 512, c0 + (h + 1) * 512)
                            ps = QPS.tile([128, 512], F32, tag="mm")
                            for ci in range(CT):
                                nc.tensor.matmul(
                                    ps[:], wt[:, ci, msl], x_sb[:, ci, hs],
                                    start=(ci == 0), stop=(ci == CT - 1))
                            nc.scalar.copy(dst[:, dsl], ps[:])
                            rsw = RPS.tile([128, 512], F32, tag="rsw")
                            nc.tensor.matmul(rsw[:], mrot[:], dst[:, dsl],
                                             start=True, stop=True)
                            t1 = RT.tile([128, XCH], F32, tag="t1")
                            nc.gpsimd.tensor_mul(
                                t1[:, 0:512], dst[:, dsl], cc[:, dsl])
                            t2 = RT.tile([128, XCH], F32, tag="t2")
                            nc.vector.scalar_tensor_tensor(
                                t2[:, 0:512], rsw[:], 1.0, ss[:, dsl],
                                op0=OP.mult, op1=OP.mult)
                            nc.vector.tensor_add(
                                dst[:, dsl], t1[:, 0:512], t2[:, 0:512])
                    # v^T then transpose to token-major
                    for h in range(XCH // 512):
                        hs = slice(h * 512, (h + 1) * 512)
                        ps = QPS.tile([128, 512], F32, tag="mm")
                        for ci in range(CT):
                            nc.tensor.matmul(
                                ps[:], wv_sb[:, ci, :], x_sb[:, ci, hs],
                                start=(ci == 0), stop=(ci == CT - 1))
                        vt_sb = VT.tile([128, 512], BF16, tag="vt")
                        nc.scalar.copy(vt_sb[:], ps[:])
                        for sub in range(4):
                            pt = TPS.tile([128, 128], BF16, tag="tr")
                            nc.tensor.transpose(
                                pt[:], vt_sb[:, sub * 128:(sub + 1) * 128],
                                identb[:])
                            j = (c0 + h * 512) // 128 + sub
                            nc.vector.tensor_copy(
                                vtok[:, j * 128:(j + 1) * 128], pt[:])

            # -------- Phases 3-6: attention -> per-batch AllToAll -> proj ----
            with tc.tile_pool(name="pp", bufs=6) as PPOOL, \
                 tc.tile_pool(name="rr", bufs=3) as RR, \
                 tc.tile_pool(name="yev", bufs=3) as YEV, \
                 tc.tile_pool(name="ya", bufs=3) as YA, \
                 tc.tile_pool(name="oev", bufs=3) as OEV, \
                 tc.tile_pool(name="wpp", bufs=1) as WPP, \
                 tc.tile_pool(name="sS", bufs=3, space="PSUM") as PS_S, \
                 tc.tile_pool(name="sAcc", bufs=2, space="PSUM") as PS_A, \
                 tc.tile_pool(name="sY", bufs=3, space="PSUM") as PS_Y:
                wp_sb = WPP.tile([128, CT, DIM], BF16, tag="wp")
                # ACT DMA ring (Q10, ~300GB/s): loads during attn b0, clear
                # of the sync ring and mostly done before the first A2A
                nc.scalar.dma_start(wp_sb[:], wpT[:])

                def proj_slice(b):
                    ya = YA.tile([128, CT, TSL], BF16, tag="ya")
                    # per-ci pieces split across the sync + vector queues
                    # (both idle here): two DMA rings in parallel, no
                    # head-of-line blocking of evictions
                    for ci in range(CT):
                        q = nc.sync if ci % 2 == 0 else nc.scalar
                        q.dma_start(ya[:, ci, :],
                                    a2aout[b][ci * 128:(ci + 1) * 128, :])
                    # ci-outer over 8 concurrent PSUM accumulation groups
                    # (banks borrowed from the attention pools, which are
                    # quiet by proj time): compute chases the ya pieces
                    # instead of waiting for the full 1MB reload
                    for og in range(2):
                        psl = [PS_S.tile([128, 512], F32, tag="S",
                                         name=f"pjS{og}_{k}") for k in range(3)]
                        psl += [PS_A.tile([128, QCH], F32, tag="sums",
                                          name=f"pjA{og}_{k}") for k in range(2)]
                        psl += [PS_Y.tile([128, QCH], F32, tag="yacc",
                                          name=f"pjY{og}_{k}") for k in range(3)]
                        for ci in range(CT):
                            for oi in range(8):
                                o = og * 8 + oi
                                nc.tensor.matmul(
                                    psl[oi][:, 0:TSL],
                                    wp_sb[:, ci, o * 128:(o + 1) * 128],
                                    ya[:, ci, :],
                                    start=(ci == 0), stop=(ci == CT - 1),
                                    skip_group_check=True)
                        for oi in range(8):
                            o = og * 8 + oi
                            oe = OEV.tile([128, TSL], BF16, tag="oe")
                            nc.scalar.copy(oe[:], psl[oi][:, 0:TSL])
                            # 2MB of 512B-line writes: alternate two HW rings
                            q = nc.sync if oi % 2 == 0 else nc.scalar
                            q.dma_start(
                                outT[o * 128:(o + 1) * 128,
                                     b * TSL:(b + 1) * TSL], oe[:])

                for b in range(B):
                    for J in range(NTCH):
                        qsl = slice(b * T + J * QCH, b * T + (J + 1) * QCH)
                        yacc = PS_Y.tile([128, QCH], F32, tag="yacc")
                        sums = PS_A.tile([128, QCH], F32, tag="sums")
                        ntk = (J + 1) * (QCH // KTILE)
                        for i in range(ntk):
                            c0 = max(0, KTILE * i - QCH * J)
                            S = PS_S.tile([128, QCH], F32, tag="S")
                            nc.tensor.matmul(
                                S[:, c0:QCH],
                                kT[:, b * T + i * KTILE: b * T + (i + 1) * KTILE],
                                qdT[:, qsl.start + c0:qsl.stop],
                                start=True, stop=True)
                            if c0 + 128 <= QCH and KTILE * i >= QCH * J:
                                nc.tensor.matmul(
                                    S[:, c0:c0 + 128], identb[:], tri[:],
                                    start=False, stop=True, skip_group_check=True)
                            P = PPOOL.tile([128, QCH], BF16, tag="P")
                            nc.scalar.activation(P[:, c0:QCH], S[:, c0:QCH], AF.Exp)
                            nc.tensor.matmul(
                                sums[:, c0:QCH], onesb[:], P[:, c0:QCH],
                                start=(i == 0), stop=(i == ntk - 1),
                                skip_group_check=True)
                            j = (b * T) // 128 + i
                            nc.tensor.matmul(
                                yacc[:, c0:QCH], vtok[:, j * 128:(j + 1) * 128],
                                P[:, c0:QCH],
                                start=(i == 0), stop=(i == ntk - 1),
                                skip_group_check=True)
                        # 1/s as exp(-ln s) on ACT: DVE reciprocal is ~9cyc/elem
                        # and would pace the whole attention pipeline
                        lns = RR.tile([128, QCH], F32, tag="lns")
                        nc.scalar.activation(lns[:], sums[:], AF.Ln)
                        rs = RR.tile([128, QCH], F32, tag="rs")
                        nc.scalar.activation(rs[:], lns[:], AF.Exp, scale=-1.0)
                        yev = YEV.tile([128, QCH], BF16, tag="ye")
                        nc.vector.scalar_tensor_tensor(
                            yev[:], yacc[:], 1.0, rs[:], op0=OP.mult, op1=OP.mult)
                        for u in range(2):
                            r0 = (2 * J + u) * 2 * HD
                            # 512B-line stores run ~25GB/s per DMA ring; use
                            # sync for dense + gpsimd for sparse (two rings in
                            # parallel, and NOT the ACT queue — a store wait
                            # there blocks the next chunk's exp and stalls PE)
                            nc.sync.dma_start(
                                a2ain[b][r0:r0 + 128, :],
                                yev[:, u * TSL:(u + 1) * TSL])

                        # ---- sparse head, same (b, J) chunk ----
                        Ssp = PS_S.tile([128, QCH], F32, tag="S")
                        nc.tensor.matmul(
                            Ssp[0:NS, :], ksT[:, b * NS:(b + 1) * NS], qsT[:, qsl],
                            start=True, stop=True)
                        nc.tensor.matmul(
                            Ssp[0:NS, :], identb[0:NS, 0:NS],
                            smask[:, J * QCH:(J + 1) * QCH],
                            start=False, stop=True, skip_group_check=True)
                        Psp = PPOOL.tile([128, QCH], BF16, tag="P")
                        nc.scalar.activation(Psp[0:NS, :], Ssp[0:NS, :], AF.Exp)
                        sums2 = PS_A.tile([128, QCH], F32, tag="sums")
                        nc.tensor.matmul(sums2[:], onesb[0:NS, :], Psp[0:NS, :],
                                         start=True, stop=True)
                        yacc2 = PS_Y.tile([128, QCH], F32, tag="yacc")
                        nc.tensor.matmul(
                            yacc2[:], vs[:, b * HD:(b + 1) * HD], Psp[0:NS, :],
                            start=True, stop=True)
                        rs2 = RR.tile([128, QCH], F32, tag="rs")
                        nc.vector.reciprocal(rs2[:], sums2[:])
                        yev2 = YEV.tile([128, QCH], BF16, tag="ye")
                        nc.vector.scalar_tensor_tensor(
                            yev2[:], yacc2[:], 1.0, rs2[:], op0=OP.mult, op1=OP.mult)
                        for u in range(2):
                            r0 = (2 * J + u) * 2 * HD + 128
                            nc.gpsimd.dma_start(
                                a2ain[b][r0:r0 + 128, :],
                                yev2[:, u * TSL:(u + 1) * TSL])
                        if J == NTCH - 1:
                            nc.gpsimd.collective_compute(
                                "AllToAll", OP.bypass,
                                ins=[a2ain[b][:]], outs=[a2aout[b][:]],
                                replica_groups=[list(range(N_CORES))],
                            )

                for b in range(B):
                    proj_slice(b)

    split_multi_waits(nc)
    return nc


_PROG_CACHE = {}


def _get_program():
    if "nc" not in _PROG_CACHE:
        _PROG_CACHE["nc"] = build_program()
    return _PROG_CACHE["nc"]


def _host_prep(x, w_attn, w_proj, q_gain, attn_temp):
    x = np.asarray(x, np.float32)
    w_attn = np.asarray(w_attn, np.float32)
    w_proj = np.asarray(w_proj, np.float32)
    q_gain = np.asarray(q_gain, np.float32)
    attn_temp = np.asarray(attn_temp, np.float32)

    BF = ml_dtypes.bfloat16

    def pack(wT):  # [DIM, M] -> [128, CT, M] partition-major (straight DMA)
        return np.ascontiguousarray(
            wT.reshape(CT, 128, -1).transpose(1, 0, 2).astype(BF))

    xT = np.ascontiguousarray(x.reshape(BT, DIM).T.astype(BF))       # [DIM, BT]
    xs = x[:, ::STRIDE, :]                                           # [B, 46, DIM]
    xsT = pack(xs.reshape(B * NS, DIM).T)                            # [128,CT,92]

    g = (q_gain * attn_temp * SCALE).astype(np.float32)              # [H]
    wq = w_attn[:H * HD].reshape(H, HD, DIM)
    wq = wq * g[:, None, None]
    wk = w_attn[H * HD:(H + KV) * HD].reshape(KV, HD, DIM)
    wv = w_attn[(H + KV) * HD:].reshape(KV, HD, DIM)

    # w_proj^T with input dims permuted to AG row order:
    # rank r contributes [dense head r | sparse head 8+r]
    perm = np.concatenate(
        [np.concatenate([np.arange(r * HD, (r + 1) * HD),
                         np.arange((8 + r) * HD, (9 + r) * HD)])
         for r in range(N_CORES)])
    wpT_bf = pack(np.ascontiguousarray(w_proj.T[perm, :]))           # [128,CT,DIM]

    in_maps = []
    for c in range(N_CORES):
        kva, kvb = c // 2, 4 + c // 2
        in_maps.append({
            "xT": xT,
            "xsT": xsT,
            "wqT": pack(np.concatenate([wq[c], wq[8 + c]], axis=0).T),
            "wkT": pack(wk[kva].T),
            "wvT": pack(wv[kva].T),
            "wksT": pack(wk[kvb].T),
            "wvsT": pack(wv[kvb].T),
            "wpT": wpT_bf,
        })
    return in_maps


def run(x, w_attn, w_proj, q_gain, attn_temp, trace=False):
    nc = _get_program()
    in_maps = _host_prep(x, w_attn, w_proj, q_gain, attn_temp)
    res = run_bass_kernel_spmd(nc, in_maps, core_ids=list(range(N_CORES)),
                               trace=trace)
    outT = np.empty((DIM, BT), np.float32)
    for c in range(N_CORES):
        sh = np.asarray(res.results[c]["outT"]).astype(np.float32)   # [DIM, B*TSL]
        for b in range(B):
            outT[:, b * T + c * TSL: b * T + (c + 1) * TSL] = \
                sh[:, b * TSL:(b + 1) * TSL]
    out = outT.T.reshape(B, T, DIM).astype(np.float32)
    return out, res


def kernel(x, w_attn, w_proj, q_gain, attn_temp):
    out, _ = run(x, w_attn, w_proj, q_gain, attn_temp, trace=False)
    return out


# revision 60
# speedup vs baseline: 1.0318x; 1.0318x over previous
"""Trainium2 Bass kernel for nn_CausalSelfAttention_77695958385275.

Self-contained: hardcodes shapes/sharding from the problem spec.

Architecture (8 NeuronCores, tensor-parallel over heads, SPMD-homogeneous):
  core c owns: dense head c, sparse head 8+c, full KV head c//2 (for the
  dense head), strided-only KV head 4+c//2 (for the sparse head).
  Every core runs the identical program; only input data differs.

v2 vs v1: bf16 attention operands (kT/q/v/P — halves LDWEIGHTS, 1c/r
masks), softmax denominators accumulated on DVE instead of per-tile
ones-matmuls (cuts 1/3 of dense-attention PE streams), Shared-output
AllToAll, warmup collective removed (b0's A2A absorbs cold-start off
the critical path), merged startup scopes, per-ci ya loads on the idle
sync queue (kills the scalar-queue head-of-line stall before the b1
projection).
"""

import math
import ml_dtypes
import numpy as np

import bass_rust
import concourse.bass as bass
import concourse.tile as tile
from concourse import mybir
from concourse.bass_utils import run_bass_kernel_spmd
from concourse.tile import TileContext

# ---------------- problem constants ----------------
B, T, DIM = 2, 2048, 2048
H, KV, HD = 16, 8, 128
NUM_FULL = 8
STRIDE = 45
NS = (T + STRIDE - 1) // STRIDE  # 46 strided keys per batch
SCALE = 1.0 / np.sqrt(np.float32(HD)).astype(np.float32)
N_CORES = 8
BT = B * T  # 4096 tokens total
HALF = HD // 2

F32 = mybir.dt.float32
F32R = mybir.dt.float32r
BF16 = mybir.dt.bfloat16

QCH = 512            # attention q-chunk width
NTCH = T // QCH      # 4 q-chunks per batch
KTILE = 128          # key tile
XCH = 1024           # qkv token chunk (2KB DMA lines)
CT = DIM // 128      # 16 contraction tiles
TSL = T // N_CORES   # 256 tokens per rank per batch

ScopedClock = bass_rust.ScopedClock


class SplitDrainTileContext(TileContext):
    """This walrus build allows a single sync-wait slot per CTRL/drain;
    split the tail drain's waits across a chain of single-wait drains."""

    def _drain_and_barrier(self, tick_clock, wait_clock):
        nc = self.nc
        drain_inst = nc.sync.drain()
        wait_clock.add_sem_waits(
            drain_inst.ins, ScopedClock({None: tick_clock.global_clock})
        )
        si = drain_inst.ins.sync_info
        ow = list(si.on_wait or []) if si is not None else []
        if len(ow) > 1:
            si.on_wait = [ow[0]]
            drain_inst.ins.sync_info = si
            for w in ow[1:]:
                d2 = nc.sync.drain()
                s2 = d2.ins.sync_info
                if s2 is None:
                    s2 = bass_rust.SyncInfo(on_wait=[w], on_update=[])
                else:
                    s2.on_wait = [w]
                d2.ins.sync_info = s2
        nc.all_engine_barrier()
        assert self.sems is not None
        popped = nc._tile_sem_poison_stack.pop()
        assert popped is self._sem_poison
        nc.clear_and_free_semaphores(list(self.sems.allocated().values()))
        nc.all_engine_barrier()


def split_multi_waits(nc, max_waits=1):
    """Walrus here rejects >1 sync wait on several instruction formats; move
    extra waits onto preceding same-engine NoOps."""
    for f in nc.m.functions:
        for b in f.blocks:
            new = []
            changed = False
            for inst in b.instructions:
                si = inst.sync_info
                ow = list(si.on_wait) if (si is not None and si.on_wait) else []
                if len(ow) > max_waits:
                    changed = True
                    for w in ow[:-max_waits]:
                        nop = mybir.InstNoOp(
                            name=nc.get_next_instruction_name(), ins=[], outs=[]
                        )
                        nop.engine = inst.engine
                        nop.sync_info = bass_rust.SyncInfo(on_wait=[w], on_update=[])
                        new.append(nop)
                    si.on_wait = ow[-max_waits:]
                    inst.sync_info = si
                new.append(inst)
            if changed:
                b.instructions = new


# ---------------- host-side constant tables ----------------

def _rope_tables():
    pos = np.arange(T, dtype=np.float32)
    freqs = (np.arange(HALF, dtype=np.float32) / np.float32(HALF)).astype(np.float32)
    ang = pos[:, None] * freqs[None, :]          # [T, 64] f32
    cosv = np.cos(ang.astype(np.float64)).astype(np.float32).T   # [64, T]
    sinv = np.sin(ang.astype(np.float64)).astype(np.float32).T
    cc = np.concatenate([cosv, cosv], axis=0)    # [128, T]
    ss = np.concatenate([sinv, sinv], axis=0)
    ccT = np.concatenate([cc, cc], axis=1)       # [128, 4096] (b0|b1)
    ssT = np.concatenate([ss, ss], axis=1)
    sp = np.arange(0, T, STRIDE)
    ccS = np.concatenate([cc[:, sp], cc[:, sp]], axis=1)  # [128, 92]
    ssS = np.concatenate([ss[:, sp], ss[:, sp]], axis=1)
    return ccT, ssT, ccS, ssS


def _const_tables():
    BF = ml_dtypes.bfloat16
    ccT, ssT, ccS, ssS = _rope_tables()
    mrotT = np.zeros((HD, HD), np.float32)
    for i in range(HALF):
        mrotT[i + HALF, i] = -1.0   # (M^T)[i+64, i]: out[0:64] = -q[64:128]
        mrotT[i, i + HALF] = 1.0    # out[64:128] = +q[0:64]
    ident = np.eye(128, dtype=np.float32)
    ones = np.ones((128, 128), np.float32)
    # additive causal masks: 0 where valid, -1e9 where masked (added to
    # scores in PSUM via an identity-lhsT matmul; exp then yields 0)
    tri = np.where(np.arange(128)[None, :] >= np.arange(128)[:, None],
                   0.0, -1e9).astype(np.float32)          # [jk, x]
    q = np.arange(T)
    smask = np.where(q[None, :] >= (STRIDE * np.arange(NS))[:, None],
                     0.0, -1e9).astype(np.float32)        # [46, T]
    cast = lambda a: np.ascontiguousarray(a.astype(BF))
    return (cast(ccT), cast(ssT), cast(ccS), cast(ssS), cast(mrotT),
            cast(ident), np.ascontiguousarray(ones), cast(ones), cast(tri),
            cast(smask))


# ---------------- device program ----------------

def build_program():
    nc = bass.Bass(num_devices=N_CORES)

    # weights host-packed to [128, CT, M]: straight partition-major DMAs with
    # multi-KB contiguous lines (the [DIM, M] rearrange form had 512B lines)
    xT = nc.dram_tensor("xT", [DIM, BT], BF16, kind="ExternalInput")
    xsT = nc.dram_tensor("xsT", [128, CT, B * NS], BF16, kind="ExternalInput")
    wqT = nc.dram_tensor("wqT", [128, CT, 2 * HD], BF16, kind="ExternalInput")
    wkT = nc.dram_tensor("wkT", [128, CT, HD], BF16, kind="ExternalInput")
    wvT = nc.dram_tensor("wvT", [128, CT, HD], BF16, kind="ExternalInput")
    wksT = nc.dram_tensor("wksT", [128, CT, HD], BF16, kind="ExternalInput")
    wvsT = nc.dram_tensor("wvsT", [128, CT, HD], BF16, kind="ExternalInput")
    wpT = nc.dram_tensor("wpT", [128, CT, DIM], BF16, kind="ExternalInput")
    # token-sharded projection: each core ends up with a 256-token slice per
    # batch; host assembles by token slices
    outT = nc.dram_tensor("outT", [DIM, B * TSL], BF16, kind="ExternalOutput")
    wu_in = nc.dram_tensor("wu_in", [64, 64], BF16, kind="Internal")
    wu_out = nc.dram_tensor("wu_out", [64, 64], BF16, kind="Internal")
    wu2_in = nc.dram_tensor("wu2_in", [64, 64], BF16, kind="Internal")
    wu2_out = nc.dram_tensor("wu2_out", [64, 64], BF16, kind="Internal")

    # AllToAll per batch: in rows = 8 blocks of [dense128|sparse128] per
    # destination rank; out rows = same blocks from each source rank
    a2ain = [nc.dram_tensor(f"a2ain{b}", [N_CORES * 2 * HD, TSL], BF16,
                            kind="Internal") for b in range(B)]
    a2aout = [nc.dram_tensor(f"a2aout{b}", [N_CORES * 2 * HD, TSL], BF16,
                             kind="Internal") for b in range(B)]

    (ccT_h, ssT_h, ccS_h, ssS_h, mrotT_h, ident_h, onesf_h, onesb_h,
     tri_h, smask_h) = _const_tables()
    ccT_d = nc.inline_tensor(ccT_h, "ccT")
    ssT_d = nc.inline_tensor(ssT_h, "ssT")
    ccS_d = nc.inline_tensor(ccS_h, "ccS")
    ssS_d = nc.inline_tensor(ssS_h, "ssS")
    mrotT_d = nc.inline_tensor(mrotT_h, "mrotT")
    ident_d = nc.inline_tensor(ident_h, "ident")
    onesb_d = nc.inline_tensor(onesb_h, "onesb")
    tri_d = nc.inline_tensor(tri_h, "trim")
    smask_d = nc.inline_tensor(smask_h, "smask")

    AF = mybir.ActivationFunctionType
    OP = mybir.AluOpType

    with SplitDrainTileContext(nc) as tc:
        with tc.tile_pool(name="persist", bufs=1) as PP:
            # persistent SBUF state (bf16 except the f32r ones for sums)
            qdT = PP.tile([128, BT], BF16, tag="qdT")    # dense-head q^T (roped in place)
            qsT = PP.tile([128, BT], BF16, tag="qsT")    # sparse-head q^T
            kT = PP.tile([128, BT], BF16, tag="kT")      # full k^T of kv_a
            vtok = PP.tile([128, BT], BF16, tag="vtok")  # v token-major, 32 tiles of [128t,128d]
            ksT = PP.tile([128, B * NS], BF16, tag="ksT")     # strided k^T of kv_b
            vs = PP.tile([NS, B * HD], BF16, tag="vs")        # strided v token-major [46, (b,d)]
            mrot = PP.tile([128, 128], BF16, tag="mrot")
            identb = PP.tile([128, 128], BF16, tag="identb")
            onesb = PP.tile([128, 128], BF16, tag="onesb")
            tri = PP.tile([128, 128], BF16, tag="tri")
            smask = PP.tile([NS, T], BF16, tag="smask")
            ccS = PP.tile([128, B * NS], BF16, tag="ccS")
            ssS = PP.tile([128, B * NS], BF16, tag="ssS")

            # ------- Phase 1: strided k/v + QKV with per-chunk fused RoPE ----
            with tc.tile_pool(name="wstr", bufs=1) as WS, \
                 tc.tile_pool(name="wq", bufs=1) as WQ, \
                 tc.tile_pool(name="xs", bufs=2) as XS, \
                 tc.tile_pool(name="rtmp", bufs=3) as RT, \
                 tc.tile_pool(name="vtmp", bufs=2) as VT, \
                 tc.tile_pool(name="qkps", bufs=4, space="PSUM") as QPS, \
                 tc.tile_pool(name="trps", bufs=2, space="PSUM") as TPS, \
                 tc.tile_pool(name="rps", bufs=2, space="PSUM") as RPS:
                # startup-critical DMAs first: small weights for 1a + 1b
                xs_sb = WS.tile([128, CT, B * NS], BF16, tag="xs")
                wks_sb = WS.tile([128, CT, HD], BF16, tag="wks")
                wvs_sb = WS.tile([128, CT, HD], BF16, tag="wvs")
                wq_sb = WQ.tile([128, CT, 2 * HD], BF16, tag="wq")
                wk_sb = WQ.tile([128, CT, HD], BF16, tag="wk")
                wv_sb = WQ.tile([128, CT, HD], BF16, tag="wv")
                cc = WQ.tile([128, BT], BF16, tag="cc")
                ss = WQ.tile([128, BT], BF16, tag="ss")
                nc.sync.dma_start(wq_sb[:], wqT[:])
                nc.sync.dma_start(wk_sb[:], wkT[:])
                nc.sync.dma_start(wv_sb[:], wvT[:])
                nc.sync.dma_start(xs_sb[:], xsT[:])
                nc.sync.dma_start(wks_sb[:], wksT[:])
                nc.sync.dma_start(wvs_sb[:], wvsT[:])
                nc.sync.dma_start(mrot[:], mrotT_d[:])
                nc.sync.dma_start(ccS[:], ccS_d[:])
                nc.sync.dma_start(ssS[:], ssS_d[:])

                def phase_1a():
                    # strided k/v for the sparse kv head; emitted between QKV
                    # chunks 0 and 1 so it fills a PE bubble instead of
                    # gating the first QKV chunk on its input DMAs
                    ps_ks = QPS.tile([128, 512], F32, tag="mm", name="ps_ks")
                    for ci in range(CT):
                        nc.tensor.matmul(ps_ks[:, 0:B * NS], wks_sb[:, ci, :],
                                         xs_sb[:, ci, :],
                                         start=(ci == 0), stop=(ci == CT - 1))
                    nc.scalar.copy(ksT[:], ps_ks[:, 0:B * NS])
                    for b in range(B):
                        psv = QPS.tile([128, 512], F32, tag="mm", name="psv")
                        for ci in range(CT):
                            nc.tensor.matmul(
                                psv[0:NS, 0:HD],
                                xs_sb[:, ci, b * NS:(b + 1) * NS],
                                wvs_sb[:, ci, :],
                                start=(ci == 0), stop=(ci == CT - 1))
                        nc.vector.tensor_copy(vs[:, b * HD:(b + 1) * HD],
                                              psv[0:NS, 0:HD])
                    # strided k rope
                    rsw_sp = RPS.tile([128, 512], F32, tag="rsw", name="rsw_sp")
                    nc.tensor.matmul(rsw_sp[:, 0:B * NS], mrot[:], ksT[:],
                                     start=True, stop=True)
                    t1s = RT.tile([128, XCH], F32, tag="t1", name="t1s")
                    nc.gpsimd.tensor_mul(t1s[:, 0:B * NS], ksT[:], ccS[:])
                    t2s = RT.tile([128, XCH], F32, tag="t2", name="t2s")
                    nc.vector.scalar_tensor_tensor(
                        t2s[:, 0:B * NS], rsw_sp[:, 0:B * NS], 1.0, ssS[:],
                        op0=OP.mult, op1=OP.mult)
                    nc.vector.tensor_add(ksT[:], t1s[:, 0:B * NS],
                                         t2s[:, 0:B * NS])

                # ---- phase 1b: QKV chunk loop ----
                for tch in range(BT // XCH):
                    if tch == 1:
                        phase_1a()
                    c0 = tch * XCH
                    sl = slice(c0, c0 + XCH)
                    x_sb = XS.tile([128, CT, XCH], BF16, tag="x")
                    # per-ci pieces (2KB lines): accumulation groups chase the
                    # DMA instead of waiting for the whole chunk
                    for ci in range(CT):
                        nc.sync.dma_start(
                            x_sb[:, ci, :], xT[ci * 128:(ci + 1) * 128, sl])
                    nc.sync.dma_start(cc[:, sl], ccT_d[:, sl])
                    nc.sync.dma_start(ss[:, sl], ssT_d[:, sl])
                    if tch == 0:
                        # attention-phase constants: queue behind chunk 0 so
                        # they don't delay the startup-critical loads above
                        nc.sync.dma_start(identb[:], ident_d[:])
                        nc.sync.dma_start(onesb[:], onesb_d[:])
                        nc.sync.dma_start(tri[:], tri_d[:])
                        nc.sync.dma_start(smask[:], smask_d[:])

                    if tch == 1:
                        # tiny warmup AllToAll, triggered mid-QKV: acts as a
                        # cross-rank barrier so the first real A2A sees little
                        # rank skew; completes long before phase 1 drains
                        nc.gpsimd.collective_compute(
                            "AllToAll", OP.bypass,
                            ins=[wu_in[:]], outs=[wu_out[:]],
                            replica_groups=[list(range(N_CORES))],
                        )
                    if tch == BT // XCH - 1:
                        # second barrier, data-gated to fire near QKV end
                        # (~25us before the first real A2A): re-syncs the rank
                        # skew that re-accumulates over the 150us QKV phase
                        # and otherwise inflates the b0 A2A by 10-20us
                        nc.sync.dma_start(wu2_in[:], qdT[0:64, BT - 64:BT])
                        nc.gpsimd.collective_compute(
                            "AllToAll", OP.bypass,
                            ins=[wu2_in[:]], outs=[wu2_out[:]],
                            replica_groups=[list(range(N_CORES))],
                        )
                    # q0, q1, k -> evict -> rope in place (two 512 halves)
                    for mi, (wt, msl, dst) in enumerate((
                            (wq_sb, slice(0, 128), qdT),
                            (wq_sb, slice(128, 256), qsT),
                            (wk_sb, slice(0, 128), kT))):
                        for h in range(XCH // 512):
                            hs = slice(h * 512, (h + 1) * 512)
                            dsl = slice(c0 + h * 512, c0 + (h + 1) * 512)
                            ps = QPS.tile([128, 512], F32, tag="mm")
                            for ci in range(CT):
                                nc.tensor.matmul(
                                    ps[:], wt[:, ci, msl], x_sb[:, ci, hs],
                                    start=(ci == 0), stop=(ci == CT - 1))
                            nc.scalar.copy(dst[:, dsl], ps[:])
                            rsw = RPS.tile([128, 512], F32, tag="rsw")
                            nc.tensor.matmul(rsw[:], mrot[:], dst[:, dsl],
                                             start=True, stop=True)
                            t1 = RT.tile([128, XCH], F32, tag="t1")
                            nc.gpsimd.tensor_mul(
                                t1[:, 0:512], dst[:, dsl], cc[:, dsl])
                            t2 = RT.tile([128, XCH], F32, tag="t2")
                            nc.vector.scalar_tensor_tensor(
                                t2[:, 0:512], rsw[:], 1.0, ss[:, dsl],
                                op0=OP.mult, op1=OP.mult)
                            nc.vector.tensor_add(
                                dst[:, dsl], t1[:, 0:512], t2[:, 0:512])
                    # v^T then transpose to token-major
                    for h in range(XCH // 512):
                        hs = slice(h * 512, (h + 1) * 512)
                        ps = QPS.tile([128, 512], F32, tag="mm")
                        for ci in range(CT):
                            nc.tensor.matmul(
                                ps[:], wv_sb[:, ci, :], x_sb[:, ci, hs],
                                start=(ci == 0), stop=(ci == CT - 1))
                        vt_sb = VT.tile([128, 512], BF16, tag="vt")
                        nc.scalar.copy(vt_sb[:], ps[:])
                        for sub in range(4):
                            pt = TPS.tile([128, 128], BF16, tag="tr")
                            nc.tensor.transpose(
                                pt[:], vt_sb[:, sub * 128:(sub + 1) * 128],
                                identb[:])
                            j = (c0 + h * 512) // 128 + sub
                            nc.vector.tensor_copy(
                                vtok[:, j * 128:(j + 1) * 128], pt[:])

            # -------- Phases 3-6: attention -> per-batch AllToAll -> proj ----
            with tc.tile_pool(name="pp", bufs=6) as PPOOL, \
                 tc.tile_pool(name="rr", bufs=3) as RR, \
                 tc.tile_pool(name="yev", bufs=3) as YEV, \
                 tc.tile_pool(name="ya", bufs=3) as YA, \
                 tc.tile_pool(name="oev", bufs=3) as OEV, \
                 tc.tile_pool(name="wpp", bufs=1) as WPP, \
                 tc.tile_pool(name="sS", bufs=3, space="PSUM") as PS_S, \
                 tc.tile_pool(name="sAcc", bufs=2, space="PSUM") as PS_A, \
                 tc.tile_pool(name="sY", bufs=3, space="PSUM") as PS_Y:
                wp_sb = WPP.tile([128, CT, DIM], BF16, tag="wp")
                # ACT DMA ring (Q10, ~300GB/s): loads during attn b0, clear
                # of the sync ring and mostly done before the first A2A
                nc.scalar.dma_start(wp_sb[:], wpT[:])

                def proj_slice(b):
                    ya = YA.tile([128, CT, TSL], BF16, tag="ya")
                    # per-ci pieces split across the sync + vector queues
                    # (both idle here): two DMA rings in parallel, no
                    # head-of-line blocking of evictions
                    for ci in range(CT):
                        q = nc.sync if ci % 2 == 0 else nc.scalar
                        q.dma_start(ya[:, ci, :],
                                    a2aout[b][ci * 128:(ci + 1) * 128, :])
                    # ci-outer over 8 concurrent PSUM accumulation groups
                    # (banks borrowed from the attention pools, which are
                    # quiet by proj time): compute chases the ya pieces
                    # instead of waiting for the full 1MB reload
                    for og in range(2):
                        psl = [PS_S.tile([128, 512], F32, tag="S",
                                         name=f"pjS{og}_{k}") for k in range(3)]
                        psl += [PS_A.tile([128, QCH], F32, tag="sums",
                                          name=f"pjA{og}_{k}") for k in range(2)]
                        psl += [PS_Y.tile([128, QCH], F32, tag="yacc",
                                          name=f"pjY{og}_{k}") for k in range(3)]
                        for ci in range(CT):
                            for oi in range(8):
                                o = og * 8 + oi
                                nc.tensor.matmul(
                                    psl[oi][:, 0:TSL],
                                    wp_sb[:, ci, o * 128:(o + 1) * 128],
                                    ya[:, ci, :],
                                    start=(ci == 0), stop=(ci == CT - 1),
                                    skip_group_check=True)
                        for oi in range(8):
                            o = og * 8 + oi
                            oe = OEV.tile([128, TSL], BF16, tag="oe")
                            nc.scalar.copy(oe[:], psl[oi][:, 0:TSL])
                            # 2MB of 512B-line writes: alternate two HW rings
                            q = nc.sync if oi % 2 == 0 else nc.scalar
                            q.dma_start(
                                outT[o * 128:(o + 1) * 128,
                                     b * TSL:(b + 1) * TSL], oe[:])

                for b in range(B):
                    for J in range(NTCH):
                        qsl = slice(b * T + J * QCH, b * T + (J + 1) * QCH)
                        yacc = PS_Y.tile([128, QCH], F32, tag="yacc")
                        sums = PS_A.tile([128, QCH], F32, tag="sums")
                        ntk = (J + 1) * (QCH // KTILE)
                        for i in range(ntk):
                            c0 = max(0, KTILE * i - QCH * J)
                            S = PS_S.tile([128, QCH], F32, tag="S")
                            nc.tensor.matmul(
                                S[:, c0:QCH],
                                kT[:, b * T + i * KTILE: b * T + (i + 1) * KTILE],
                                qdT[:, qsl.start + c0:qsl.stop],
                                start=True, stop=True)
                            if c0 + 128 <= QCH and KTILE * i >= QCH * J:
                                nc.tensor.matmul(
                                    S[:, c0:c0 + 128], identb[:], tri[:],
                                    start=False, stop=True, skip_group_check=True)
                            P = PPOOL.tile([128, QCH], BF16, tag="P")
                            nc.scalar.activation(P[:, c0:QCH], S[:, c0:QCH], AF.Exp)
                            nc.tensor.matmul(
                                sums[:, c0:QCH], onesb[:], P[:, c0:QCH],
                                start=(i == 0), stop=(i == ntk - 1),
                                skip_group_check=True)
                            j = (b * T) // 128 + i
                            nc.tensor.matmul(
                                yacc[:, c0:QCH], vtok[:, j * 128:(j + 1) * 128],
                                P[:, c0:QCH],
                                start=(i == 0), stop=(i == ntk - 1),
                                skip_group_check=True)
                        # 1/s as exp(-ln s) on ACT: DVE reciprocal is ~9cyc/elem
                        # and would pace the whole attention pipeline
                        lns = RR.tile([128, QCH], F32, tag="lns")
                        nc.scalar.activation(lns[:], sums[:], AF.Ln)
                        rs = RR.tile([128, QCH], F32, tag="rs")
                        nc.scalar.activation(rs[:], lns[:], AF.Exp, scale=-1.0)
                        yev = YEV.tile([128, QCH], BF16, tag="ye")
                        nc.vector.scalar_tensor_tensor(
                            yev[:], yacc[:], 1.0, rs[:], op0=OP.mult, op1=OP.mult)
                        for u in range(2):
                            r0 = (2 * J + u) * 2 * HD
                            # 512B-line stores run ~25GB/s per DMA ring; use
                            # sync for dense + gpsimd for sparse (two rings in
                            # parallel, and NOT the ACT queue — a store wait
                            # there blocks the next chunk's exp and stalls PE)
                            nc.sync.dma_start(
                                a2ain[b][r0:r0 + 128, :],
                                yev[:, u * TSL:(u + 1) * TSL])

                        # ---- sparse head, same (b, J) chunk ----
                        Ssp = PS_S.tile([128, QCH], F32, tag="S")
                        nc.tensor.matmul(
                            Ssp[0:NS, :], ksT[:, b * NS:(b + 1) * NS], qsT[:, qsl],
                            start=True, stop=True)
                        nc.tensor.matmul(
                            Ssp[0:NS, :], identb[0:NS, 0:NS],
                            smask[:, J * QCH:(J + 1) * QCH],
                            start=False, stop=True, skip_group_check=True)
                        Psp = PPOOL.tile([128, QCH], BF16, tag="P")
                        nc.scalar.activation(Psp[0:NS, :], Ssp[0:NS, :], AF.Exp)
                        sums2 = PS_A.tile([128, QCH], F32, tag="sums")
                        nc.tensor.matmul(sums2[:], onesb[0:NS, :], Psp[0:NS, :],
                                         start=True, stop=True)
                        yacc2 = PS_Y.tile([128, QCH], F32, tag="yacc")
                        nc.tensor.matmul(
                            yacc2[:], vs[:, b * HD:(b + 1) * HD], Psp[0:NS, :],
                            start=True, stop=True)
                        rs2 = RR.tile([128, QCH], F32, tag="rs")
                        nc.vector.reciprocal(rs2[:], sums2[:])
                        yev2 = YEV.tile([128, QCH], BF16, tag="ye")
                        nc.vector.scalar_tensor_tensor(
                            yev2[:], yacc2[:], 1.0, rs2[:], op0=OP.mult, op1=OP.mult)
                        for u in range(2):
                            r0 = (2 * J + u) * 2 * HD + 128
                            nc.gpsimd.dma_start(
                                a2ain[b][r0:r0 + 128, :],
                                yev2[:, u * TSL:(u + 1) * TSL])
                        if J == NTCH - 1:
                            nc.gpsimd.collective_compute(
                                "AllToAll", OP.bypass,
                                ins=[a2ain[b][:]], outs=[a2aout[b][:]],
                                replica_groups=[list(range(N_CORES))],
                            )

                for b in range(B):
                    proj_slice(b)

    split_multi_waits(nc)
    return nc


_PROG_CACHE = {}


def _get_program():
    if "nc" not in _PROG_CACHE:
        _PROG_CACHE["nc"] = build_program()
    return _PROG_CACHE["nc"]


def _host_prep(x, w_attn, w_proj, q_gain, attn_temp):
    x = np.asarray(x, np.float32)
    w_attn = np.asarray(w_attn, np.float32)
    w_proj = np.asarray(w_proj, np.float32)
    q_gain = np.asarray(q_gain, np.float32)
    attn_temp = np.asarray(attn_temp, np.float32)

    BF = ml_dtypes.bfloat16

    def pack(wT):  # [DIM, M] -> [128, CT, M] partition-major (straight DMA)
        return np.ascontiguousarray(
            wT.reshape(CT, 128, -1).transpose(1, 0, 2).astype(BF))

    xT = np.ascontiguousarray(x.reshape(BT, DIM).T.astype(BF))       # [DIM, BT]
    xs = x[:, ::STRIDE, :]                                           # [B, 46, DIM]
    xsT = pack(xs.reshape(B * NS, DIM).T)                            # [128,CT,92]

    g = (q_gain * attn_temp * SCALE).astype(np.float32)              # [H]
    wq = w_attn[:H * HD].reshape(H, HD, DIM)
    wq = wq * g[:, None, None]
    wk = w_attn[H * HD:(H + KV) * HD].reshape(KV, HD, DIM)
    wv = w_attn[(H + KV) * HD:].reshape(KV, HD, DIM)

    # w_proj^T with input dims permuted to AG row order:
    # rank r contributes [dense head r | sparse head 8+r]
    perm = np.concatenate(
        [np.concatenate([np.arange(r * HD, (r + 1) * HD),
                         np.arange((8 + r) * HD, (9 + r) * HD)])
         for r in range(N_CORES)])
    wpT_bf = pack(np.ascontiguousarray(w_proj.T[perm, :]))           # [128,CT,DIM]

    in_maps = []
    for c in range(N_CORES):
        kva, kvb = c // 2, 4 + c // 2
        in_maps.append({
            "xT": xT,
            "xsT": xsT,
            "wqT": pack(np.concatenate([wq[c], wq[8 + c]], axis=0).T),
            "wkT": pack(wk[kva].T),
            "wvT": pack(wv[kva].T),
            "wksT": pack(wk[kvb].T),
            "wvsT": pack(wv[kvb].T),
            "wpT": wpT_bf,
        })
    return in_maps


def run(x, w_attn, w_proj, q_gain, attn_temp, trace=False):
    nc = _get_program()
    in_maps = _host_prep(x, w_attn, w_proj, q_gain, attn_temp)
    res = run_bass_kernel_spmd(nc, in_maps, core_ids=list(range(N_CORES)),
                               trace=trace)
    outT = np.empty((DIM, BT), np.float32)
    for c in range(N_CORES):
        sh = np.asarray(res.results[c]["outT"]).astype(np.float32)   # [DIM, B*TSL]
        for b in range(B):
            outT[:, b * T + c * TSL: b * T + (c + 1) * TSL] = \
                sh[:, b * TSL:(b + 1) * TSL]
    out = outT.T.reshape(B, T, DIM).astype(np.float32)
    return out, res


def kernel(x, w_attn, w_proj, q_gain, attn_temp):
    out, _ = run(x, w_attn, w_proj, q_gain, attn_temp, trace=False)
    return out
